# revision 7
# baseline (speedup 1.0000x reference)
"""Trainium2 Bass kernel for DirectedGraphLearner (topk_masking).

One NeuronCore per batch b (8 cores total):
    src = x_b @ W_src        [1024, 256] -> heads [4, 64]
    tgt = x_b @ W_tgt
    adj[h] = src_h @ tgt_h^T [1024, 1024]
    out[h] = gelu(adj) * topk_mask(gelu(adj), k=153, rowwise)

Algorithm (v4), exploiting that the row-wise top-k threshold lands at
adj ~ 5..13 sigma where exact-erf gelu(x) == x in fp32, so gelu never
needs computing and only positives can be kept:

  * All matmuls run as float32r (TF32-like, 4x PE throughput; rel err
    ~1.5e-4, which only perturbs ranking within ~0.004 of the
    threshold -- well inside the rel-err budget).
  * The PSUM->SBUF copy applies Relu and a free accum_out, giving
    s+ = sum(relu(adj)) per row.  For near-gaussian rows the top-k
    threshold satisfies t ~= C_T * s+ within +-12%, so a per-row
    bracket [t^(1-DLO), t^(1+DHI)] replaces the fixed [4,16) bracket.
  * 1 + NB=5 exact counts on q = bf16(relu(adj)) (DVE 4x-mode
    tensor_scalar+accum at 327ns, some lanes on ACT Sign-count / Pool)
    bisect to a bracket holding ~3 candidates; counts are exact since
    trial points are generic f32 values (never on the bf16 grid).
  * Phase 2 needs no window mask: o = [q < hi]*g keeps every value
    below the bracket top, whose r-th largest (r = K - #{q >= hi} <= 8)
    is exactly the k-th largest of the row (bf16 rounding is monotone,
    so the q-mask never reorders across the f32 boundary).  One max8 +
    iota rank-select yields the exact f32 threshold.
  * Output values are bf16-rounded (DRAM out is bf16, host upcasts);
    support is f32-exact.  Finals and the o-mult run on the otherwise
    idle Pool engine.
"""

import numpy as np

import concourse.bass as bass
from concourse import bacc
import concourse.mybir as mybir
import concourse.tile as tile
from concourse.bass_utils import run_bass_kernel_spmd

F32 = mybir.dt.float32
F32R = mybir.dt.float32r
BF16 = mybir.dt.bfloat16
ALU = mybir.AluOpType
AF = mybir.ActivationFunctionType

B, N, D, H, HD = 8, 1024, 256, 4, 64
K = 153  # max(1, int(0.15 * 1024))
NCH = N // 128  # row chunks per head

# t ~= C_T * sum(relu(row)); empirical ratio spread is -10.2%/+11.8%,
# margins widened ~1.15x.
C_T = 2.548730e-03
DLO = 0.118
DHI = 0.136
NB = 5  # bisection iterations after the bracket-top count

# count-engine assignment per (head, iter): iter -1 is the bracket-top
# count; others 0..NB-1.  Default engine is DVE; Pool cannot count
# (neuronxcc rejects TensorScalarPtr on Pool).
ACT_ITERS = {(h, it) for h in range(4) for it in (1, 3)}
POOL_ITERS = set()

_CACHED_NC = None


def _build_nc():
    nc = bacc.Bacc()
    # xb is passed host-side pre-transposed: [D, N] == x[b].T
    xb = nc.declare_dram_parameter("xb", [D, N], F32, isOutput=False)
    ws = nc.declare_dram_parameter("ws", [D, D], F32, isOutput=False)
    wt = nc.declare_dram_parameter("wt", [D, D], F32, isOutput=False)
    out = nc.declare_dram_parameter("out", [H, N, N], BF16, isOutput=True)
    with tile.TileContext(nc) as tc:
        _body(tc, xb, ws, wt, out)
    nc.compile()
    return nc


def _body(tc, xb, ws, wt, out):
    nc = tc.nc
    with (
        tc.tile_pool(name="persist", bufs=1) as ppool,
        tc.tile_pool(name="g", bufs=2) as gpool,
        tc.tile_pool(name="q", bufs=2) as qpool,
        tc.tile_pool(name="o", bufs=3) as opool,
        tc.tile_pool(name="ob", bufs=6) as obpool,
        tc.tile_pool(name="small", bufs=2) as spool,
        tc.tile_pool(name="jnk", bufs=2) as jpool,
        tc.tile_pool(name="ppsum", bufs=2, space="PSUM") as ppsum,
        tc.tile_pool(name="apsum", bufs=3, space="PSUM") as apsum,
    ):
        # ---- load xT [256, 1024] (host passes x[b].T) and weights ----
        xT = [ppool.tile([128, N], F32, tag=f"xT{d}", name=f"xT{d}") for d in range(2)]
        for dh in range(2):
            nc.sync.dma_start(xT[dh], xb[dh * 128 : (dh + 1) * 128, :])
        wst = [ppool.tile([128, D], F32, tag=f"ws{kc}", name=f"wst{kc}") for kc in range(2)]
        wtt = [ppool.tile([128, D], F32, tag=f"wt{kc}", name=f"wtt{kc}") for kc in range(2)]
        for kc in range(2):
            nc.sync.dma_start(wst[kc], ws[kc * 128 : (kc + 1) * 128, :])
            nc.sync.dma_start(wtt[kc], wt[kc * 128 : (kc + 1) * 128, :])

        # ---- projections: srcT/tgtT = (x @ W)^T = W^T x^T, [256, 1024]
        srcT = [ppool.tile([128, N], F32, tag=f"sT{m}", name=f"srcT{m}") for m in range(2)]
        tgtT = [ppool.tile([128, N], F32, tag=f"tT{m}", name=f"tgtT{m}") for m in range(2)]
        for wtiles, ttiles in ((wst, srcT), (wtt, tgtT)):
            for m in range(2):
                for nh in range(2):
                    pp = ppsum.tile([128, 512], F32, tag="pp")
                    for kc in range(2):
                        nc.tensor.matmul(
                            pp,
                            wtiles[kc][:, m * 128 : (m + 1) * 128],
                            xT[kc][:, nh * 512 : (nh + 1) * 512],
                            start=(kc == 0),
                            stop=(kc == 1),
                        )
                    nc.scalar.copy(ttiles[m][:, nh * 512 : (nh + 1) * 512], pp)

        # iota row 0..7, for rank-select from the max8 output
        iota8 = ppool.tile([128, 8], F32, tag="iota8", name="iota8")
        for j in range(8):
            nc.vector.memset(iota8[:, j : j + 1], float(j))

        # ---- per head: adj chunks, threshold search, mask, store ----
        for h in range(H):
            ht = h // 2
            hs = (h % 2) * HD
            sp = spool.tile([128, NCH], F32, tag="sp")
            gts, qts = [], []
            for i in range(NCH):
                ap = apsum.tile([128, N], F32, tag="ap")
                for nh in range(2):
                    nc.tensor.matmul(
                        ap[:, nh * 512 : (nh + 1) * 512],
                        srcT[ht][hs : hs + HD, i * 128 : (i + 1) * 128],
                        tgtT[ht][hs : hs + HD, nh * 512 : (nh + 1) * 512],
                    )
                g = gpool.tile([128, N], F32, tag=f"g{i}", name=f"g{i}")
                nc.scalar.activation(g, ap, AF.Relu, accum_out=sp[:, i : i + 1])
                gts.append(g)
                q = qpool.tile([128, N], BF16, tag=f"q{i}", name=f"q{i}")
                nc.gpsimd.tensor_copy(q, g)
                qts.append(q)

            # bracket init: lo = t^(1-DLO), w0h = half-width
            lo = spool.tile([128, NCH], F32, tag="lo")
            w0h = spool.tile([128, NCH], F32, tag="w0h")
            hi = spool.tile([128, NCH], F32, tag="hi")
            tri = spool.tile([128, NCH], F32, tag="tri")
            trin = spool.tile([128, NCH], F32, tag="trin")
            cnt = spool.tile([128, NCH], F32, tag="cnt")
            chi = spool.tile([128, NCH], F32, tag="chi")
            pred = spool.tile([128, NCH], F32, tag="pred")
            npred = spool.tile([128, NCH], F32, tag="npred")
            dl = spool.tile([128, NCH], F32, tag="dl")
            that = spool.tile([128, NCH], F32, tag="that")

            nc.vector.tensor_scalar(that, sp, float(C_T), None, op0=ALU.mult)
            nc.vector.tensor_scalar(lo, that, float(1.0 - DLO), None, op0=ALU.mult)
            nc.vector.tensor_scalar(w0h, that, float((DLO + DHI) / 2.0), None, op0=ALU.mult)
            # bracket top count: chi = #{q >= hi0}, hi0 = lo + 2*w0h
            nc.vector.scalar_tensor_tensor(hi, w0h, 2.0, lo, op0=ALU.mult, op1=ALU.add)
            for i in range(NCH):
                jk = jpool.tile([128, N], BF16, tag=f"jk{i}", name=f"jk{i}")
                nc.vector.tensor_scalar(
                    jk, qts[i], hi[:, i : i + 1], None,
                    op0=ALU.is_ge, op1=ALU.add, accum_out=chi[:, i : i + 1],
                )
            for it in range(NB):
                nc.vector.tensor_add(tri, lo, w0h)
                eng = "dve"
                if (h, it) in ACT_ITERS:
                    eng = "act"
                    nc.vector.tensor_scalar(trin, tri, -1.0, None, op0=ALU.mult)
                elif (h, it) in POOL_ITERS:
                    eng = "pool"
                for i in range(NCH):
                    jk = jpool.tile([128, N], BF16, tag=f"jk{i}", name=f"jkb{i}")
                    if eng == "act":
                        # s = sum(sign(q - tri)) = 2*cnt - N (no zeros: tri
                        # is a generic f32, never on the bf16 grid)
                        nc.scalar.activation(
                            jk, qts[i], AF.Sign,
                            bias=trin[:, i : i + 1], accum_out=cnt[:, i : i + 1],
                        )
                    else:
                        e = nc.vector if eng == "dve" else nc.gpsimd
                        e.tensor_scalar(
                            jk, qts[i], tri[:, i : i + 1], None,
                            op0=ALU.is_ge, op1=ALU.add, accum_out=cnt[:, i : i + 1],
                        )
                if eng == "act":
                    nc.vector.tensor_scalar(cnt, cnt, 0.5, float(N / 2.0), op0=ALU.mult, op1=ALU.add)
                nc.vector.tensor_scalar(pred, cnt, float(K), None, op0=ALU.is_ge)
                nc.vector.tensor_scalar(npred, pred, -1.0, 1.0, op0=ALU.mult, op1=ALU.add)
                nc.vector.tensor_mul(dl, pred, w0h)
                nc.vector.tensor_add(lo, lo, dl)
                # chi <- pred ? chi : cnt
                nc.vector.tensor_sub(dl, cnt, chi)
                nc.vector.tensor_mul(dl, dl, npred)
                nc.vector.tensor_add(chi, chi, dl)
                nc.vector.tensor_scalar(w0h, w0h, 0.5, None, op0=ALU.mult)
            nc.vector.scalar_tensor_tensor(hi, w0h, 2.0, lo, op0=ALU.mult, op1=ALU.add)

            # rank within candidates: m1 = clip(K-1 - chi, 0, 7)
            m1 = spool.tile([128, NCH], F32, tag="m1")
            tf = spool.tile([128, NCH], F32, tag="tf")
            nc.vector.tensor_scalar(m1, chi, -1.0, float(K - 1), op0=ALU.mult, op1=ALU.add)
            nc.vector.tensor_scalar_min(m1, m1, 7.0)
            nc.vector.tensor_scalar_max(m1, m1, 0.0)

            mxall = spool.tile([128, 8 * NCH], F32, tag="mxall")
            for i in range(NCH):
                # o = [q < hi] * g  (keeps all values below the bracket top)
                o = opool.tile([128, N], F32, tag="o", name=f"o{h}_{i}")
                nc.vector.scalar_tensor_tensor(
                    o, qts[i], hi[:, i : i + 1], gts[i], op0=ALU.is_lt, op1=ALU.mult
                )
                nc.vector.max(out=mxall[:, 8 * i : 8 * i + 8], in_=o)

            # batched rank-select: tf_i = mxall[i*8 + m1_i]
            selall = spool.tile([128, 8 * NCH], F32, tag="selall")
            nc.vector.tensor_tensor(
                out=selall.rearrange("p (c f) -> p c f", f=8),
                in0=m1.rearrange("p (c u) -> p c u", u=1).to_broadcast([128, NCH, 8]),
                in1=iota8.rearrange("p (u f) -> p u f", u=1).to_broadcast([128, NCH, 8]),
                op=ALU.is_equal,
            )
            nc.vector.tensor_tensor(out=selall, in0=selall, in1=mxall, op=ALU.mult)
            nc.vector.tensor_reduce(
                out=tf,
                in_=selall.rearrange("p (c f) -> p c f", f=8),
                axis=mybir.AxisListType.X,
                op=ALU.add,
            )

            for i in range(NCH):
                # final: f32-exact mask on DVE (2x mode), mult on Pool
                msk = opool.tile([128, N], F32, tag="msk", name=f"msk{h}_{i}")
                nc.vector.tensor_scalar(
                    msk, gts[i], tf[:, i : i + 1], None, op0=ALU.is_ge
                )
                ob = obpool.tile([128, N], BF16, tag="ob", name=f"ob{h}_{i}")
                nc.gpsimd.tensor_tensor(out=ob, in0=msk, in1=gts[i], op=ALU.mult)
                nc.sync.dma_start(out[h, i * 128 : (i + 1) * 128, :], ob)


def _get_nc():
    global _CACHED_NC
    if _CACHED_NC is None:
        _CACHED_NC = _build_nc()
    return _CACHED_NC


def run(x, W_src, W_tgt, trace=False):
    x = np.ascontiguousarray(np.asarray(x, dtype=np.float32))
    W_src = np.ascontiguousarray(np.asarray(W_src, dtype=np.float32))
    W_tgt = np.ascontiguousarray(np.asarray(W_tgt, dtype=np.float32))
    nc = _get_nc()
    in_maps = [
        {"xb": np.ascontiguousarray(x[b].T), "ws": W_src, "wt": W_tgt}
        for b in range(B)
    ]
    res = run_bass_kernel_spmd(nc, in_maps, list(range(B)), trace=trace)
    out = np.stack([res.results[b]["out"] for b in range(B)], axis=0).astype(np.float32)
    return out, res


def kernel(x, W_src, W_tgt):
    out, _ = run(x, W_src, W_tgt, trace=False)
    return out


# revision 8
# speedup vs baseline: 1.1635x; 1.1635x over previous
"""Trainium2 Bass kernel for DirectedGraphLearner (topk_masking).

One NeuronCore per batch b (8 cores total):
    src = x_b @ W_src        [1024, 256] -> heads [4, 64]
    tgt = x_b @ W_tgt
    adj[h] = src_h @ tgt_h^T [1024, 1024]
    out[h] = gelu(adj) * topk_mask(gelu(adj), k=153, rowwise)

Algorithm (v4), exploiting that the row-wise top-k threshold lands at
adj ~ 5..13 sigma where exact-erf gelu(x) == x in fp32, so gelu never
needs computing and only positives can be kept:

  * The PSUM->SBUF copy applies Relu and a free accum_out, giving
    s+ = sum(relu(adj)) per row.  For near-gaussian rows the top-k
    threshold satisfies t ~= C_T * s+ within +-12%, so a per-row
    bracket [t^(1-DLO), t^(1+DHI)] replaces a fixed one.
  * 1 + NB exact counts on q = bf16(relu(adj)) (DVE 4x-mode
    tensor_scalar+accum at 327ns; two chunk-lanes per head count on ACT
    via Sign+accum) bisect to a bracket holding ~3 candidates; counts
    are exact because trial points are generic f32 values that never
    land on the bf16 grid.
  * Phase 2 needs no window mask: o = [q < hi]*g keeps every value
    below the bracket top, whose r-th largest (r = K - #{q >= hi} <= 8)
    is exactly the k-th largest of the row (bf16 rounding is monotone,
    so the q-mask never splits f32-adjacent values across hi).  One
    max8 + iota rank-select yields the exact f32 threshold.
  * Output support is f32-exact; output values are bf16-rounded (DRAM
    out is bf16, host upcasts).  Casts and final mask-mults run on the
    otherwise idle Pool engine; heads are software-pipelined so the
    produce stage of head h+1 (matmul, copy, cast) is queued ahead of
    head h's search, keeping Pool's in-order queue from blocking casts.
"""

import numpy as np

import concourse.bass as bass
from concourse import bacc
import concourse.mybir as mybir
import concourse.tile as tile
from concourse.bass_utils import run_bass_kernel_spmd

F32 = mybir.dt.float32
BF16 = mybir.dt.bfloat16
ALU = mybir.AluOpType
AF = mybir.ActivationFunctionType

B, N, D, H, HD = 8, 1024, 256, 4, 64
K = 153  # max(1, int(0.15 * 1024))
NCH = N // 128  # row chunks per head

# t ~= C_T * sum(relu(row)); empirical ratio spread -10.2%/+11.8%,
# margins widened ~1.15x.
C_T = 2.548730e-03
DLO = 0.118
DHI = 0.136
NB = 5  # bisection iterations after the bracket-top count

ACT_CHUNKS = (6, 7)  # chunk lanes whose counts run on ACT (Sign+accum)

_CACHED_NC = None


def _build_nc():
    nc = bacc.Bacc()
    # xb is passed host-side pre-transposed: [D, N] == x[b].T
    xb = nc.declare_dram_parameter("xb", [D, N], F32, isOutput=False)
    ws = nc.declare_dram_parameter("ws", [D, D], F32, isOutput=False)
    wt = nc.declare_dram_parameter("wt", [D, D], F32, isOutput=False)
    out = nc.declare_dram_parameter("out", [H, N, N], BF16, isOutput=True)
    with tile.TileContext(nc) as tc:
        _body(tc, xb, ws, wt, out)
    nc.compile()
    return nc


def _body(tc, xb, ws, wt, out):
    nc = tc.nc
    with (
        tc.tile_pool(name="persist", bufs=1) as ppool,
        tc.tile_pool(name="g", bufs=2) as gpool,
        tc.tile_pool(name="q", bufs=2) as qpool,
        tc.tile_pool(name="o", bufs=3) as opool,
        tc.tile_pool(name="ob", bufs=6) as obpool,
        tc.tile_pool(name="small", bufs=2) as spool,
        tc.tile_pool(name="jnk", bufs=2) as jpool,
        tc.tile_pool(name="ppsum", bufs=2, space="PSUM") as ppsum,
        tc.tile_pool(name="apsum", bufs=3, space="PSUM") as apsum,
    ):
        # ---- load xT [256, 1024] (host passes x[b].T) and weights ----
        xT = [ppool.tile([128, N], F32, tag=f"xT{d}", name=f"xT{d}") for d in range(2)]
        for dh in range(2):
            nc.sync.dma_start(xT[dh], xb[dh * 128 : (dh + 1) * 128, :])
        wst = [ppool.tile([128, D], F32, tag=f"ws{kc}", name=f"wst{kc}") for kc in range(2)]
        wtt = [ppool.tile([128, D], F32, tag=f"wt{kc}", name=f"wtt{kc}") for kc in range(2)]
        for kc in range(2):
            nc.sync.dma_start(wst[kc], ws[kc * 128 : (kc + 1) * 128, :])
            nc.sync.dma_start(wtt[kc], wt[kc * 128 : (kc + 1) * 128, :])

        # ---- projections: srcT/tgtT = (x @ W)^T = W^T x^T, [256, 1024]
        srcT = [ppool.tile([128, N], F32, tag=f"sT{m}", name=f"srcT{m}") for m in range(2)]
        tgtT = [ppool.tile([128, N], F32, tag=f"tT{m}", name=f"tgtT{m}") for m in range(2)]
        for wtiles, ttiles in ((wst, srcT), (wtt, tgtT)):
            for m in range(2):
                for nh in range(2):
                    pp = ppsum.tile([128, 512], F32, tag="pp")
                    for kc in range(2):
                        nc.tensor.matmul(
                            pp,
                            wtiles[kc][:, m * 128 : (m + 1) * 128],
                            xT[kc][:, nh * 512 : (nh + 1) * 512],
                            start=(kc == 0),
                            stop=(kc == 1),
                        )
                    nc.scalar.copy(ttiles[m][:, nh * 512 : (nh + 1) * 512], pp)

        # iota row 0..7, for rank-select from the max8 output
        iota8 = ppool.tile([128, 8], F32, tag="iota8", name="iota8")
        for j in range(8):
            nc.vector.memset(iota8[:, j : j + 1], float(j))

        def produce(h):
            """adj matmuls + relu-copy (+accum) + bf16 cast for head h."""
            ht = h // 2
            hs = (h % 2) * HD
            sp = spool.tile([128, NCH], F32, tag=f"sp{h % 2}")
            gts, qts = [], []
            for i in range(NCH):
                ap = apsum.tile([128, N], F32, tag="ap")
                for nh in range(2):
                    nc.tensor.matmul(
                        ap[:, nh * 512 : (nh + 1) * 512],
                        srcT[ht][hs : hs + HD, i * 128 : (i + 1) * 128],
                        tgtT[ht][hs : hs + HD, nh * 512 : (nh + 1) * 512],
                    )
                g = gpool.tile([128, N], F32, tag=f"g{i}", name=f"g{h}_{i}")
                nc.scalar.activation(g, ap, AF.Relu, accum_out=sp[:, i : i + 1])
                gts.append(g)
                q = qpool.tile([128, N], BF16, tag=f"q{i}", name=f"q{h}_{i}")
                nc.gpsimd.tensor_copy(q, g)
                qts.append(q)
            return sp, gts, qts

        def search(h, sp, gts, qts):
            """bisection search + extraction + masked store for head h."""
            s = h % 2
            lo = spool.tile([128, NCH], F32, tag=f"lo{s}")
            w0h = spool.tile([128, NCH], F32, tag=f"w0h{s}")
            hi = spool.tile([128, NCH], F32, tag=f"hi{s}")
            tri = spool.tile([128, NCH], F32, tag=f"tri{s}")
            trin = spool.tile([128, NCH], F32, tag=f"trin{s}")
            cnt = spool.tile([128, NCH], F32, tag=f"cnt{s}")
            chi = spool.tile([128, NCH], F32, tag=f"chi{s}")
            pred = spool.tile([128, NCH], F32, tag=f"pred{s}")
            npred = spool.tile([128, NCH], F32, tag=f"npred{s}")
            dl = spool.tile([128, NCH], F32, tag=f"dl{s}")
            that = spool.tile([128, NCH], F32, tag=f"that{s}")

            nc.vector.tensor_scalar(that, sp, float(C_T), None, op0=ALU.mult)
            nc.vector.tensor_scalar(lo, that, float(1.0 - DLO), None, op0=ALU.mult)
            nc.vector.tensor_scalar(w0h, that, float((DLO + DHI) / 2.0), None, op0=ALU.mult)
            # bracket top: hi0 = lo + 2*w0h; chi = #{q >= hi0}
            nc.vector.scalar_tensor_tensor(hi, w0h, 2.0, lo, op0=ALU.mult, op1=ALU.add)
            nc.vector.tensor_scalar(trin, hi, -1.0, None, op0=ALU.mult)
            for i in range(NCH):
                jk = jpool.tile([128, N], BF16, tag=f"jk{i}", name=f"jkc{h}_{i}")
                if i in ACT_CHUNKS:
                    nc.scalar.activation(
                        jk, qts[i], AF.Sign,
                        bias=trin[:, i : i + 1], accum_out=chi[:, i : i + 1],
                    )
                else:
                    nc.vector.tensor_scalar(
                        jk, qts[i], hi[:, i : i + 1], None,
                        op0=ALU.is_ge, op1=ALU.add, accum_out=chi[:, i : i + 1],
                    )
            # ACT lanes return s = 2*cnt - N; convert those lanes only
            a0 = ACT_CHUNKS[0]
            nc.vector.tensor_scalar(
                chi[:, a0 : a0 + len(ACT_CHUNKS)],
                chi[:, a0 : a0 + len(ACT_CHUNKS)],
                0.5, float(N / 2.0), op0=ALU.mult, op1=ALU.add,
            )
            for it in range(NB):
                nc.vector.tensor_add(tri, lo, w0h)
                nc.vector.tensor_scalar(trin, tri, -1.0, None, op0=ALU.mult)
                for i in range(NCH):
                    jk = jpool.tile([128, N], BF16, tag=f"jk{i}", name=f"jkb{h}_{it}_{i}")
                    if i in ACT_CHUNKS:
                        nc.scalar.activation(
                            jk, qts[i], AF.Sign,
                            bias=trin[:, i : i + 1], accum_out=cnt[:, i : i + 1],
                        )
                    else:
                        nc.vector.tensor_scalar(
                            jk, qts[i], tri[:, i : i + 1], None,
                            op0=ALU.is_ge, op1=ALU.add, accum_out=cnt[:, i : i + 1],
                        )
                nc.vector.tensor_scalar(
                    cnt[:, a0 : a0 + len(ACT_CHUNKS)],
                    cnt[:, a0 : a0 + len(ACT_CHUNKS)],
                    0.5, float(N / 2.0), op0=ALU.mult, op1=ALU.add,
                )
                nc.vector.tensor_scalar(pred, cnt, float(K), None, op0=ALU.is_ge)
                nc.vector.tensor_scalar(npred, pred, -1.0, 1.0, op0=ALU.mult, op1=ALU.add)
                nc.vector.tensor_mul(dl, pred, w0h)
                nc.vector.tensor_add(lo, lo, dl)
                # chi <- pred ? chi : cnt
                nc.vector.tensor_sub(dl, cnt, chi)
                nc.vector.tensor_mul(dl, dl, npred)
                nc.vector.tensor_add(chi, chi, dl)
                nc.vector.tensor_scalar(w0h, w0h, 0.5, None, op0=ALU.mult)
            nc.vector.scalar_tensor_tensor(hi, w0h, 2.0, lo, op0=ALU.mult, op1=ALU.add)

            # rank among candidates: m1 = clip(K-1 - chi, 0, 7)
            m1 = spool.tile([128, NCH], F32, tag=f"m1{s}")
            tf = spool.tile([128, NCH], F32, tag=f"tf{s}")
            nc.vector.tensor_scalar(m1, chi, -1.0, float(K - 1), op0=ALU.mult, op1=ALU.add)
            nc.vector.tensor_scalar_min(m1, m1, 7.0)
            nc.vector.tensor_scalar_max(m1, m1, 0.0)

            mxall = spool.tile([128, 8 * NCH], F32, tag=f"mxall{s}")
            for i in range(NCH):
                # o = [q < hi] * g  (all values below the bracket top)
                o = opool.tile([128, N], F32, tag="o", name=f"o{h}_{i}")
                nc.vector.scalar_tensor_tensor(
                    o, qts[i], hi[:, i : i + 1], gts[i], op0=ALU.is_lt, op1=ALU.mult
                )
                nc.vector.max(out=mxall[:, 8 * i : 8 * i + 8], in_=o)

            # batched rank-select: tf_i = mxall[i*8 + m1_i]
            selall = spool.tile([128, 8 * NCH], F32, tag=f"selall{s}")
            nc.vector.tensor_tensor(
                out=selall.rearrange("p (c f) -> p c f", f=8),
                in0=m1.rearrange("p (c u) -> p c u", u=1).to_broadcast([128, NCH, 8]),
                in1=iota8.rearrange("p (u f) -> p u f", u=1).to_broadcast([128, NCH, 8]),
                op=ALU.is_equal,
            )
            nc.vector.tensor_tensor(out=selall, in0=selall, in1=mxall, op=ALU.mult)
            nc.vector.tensor_reduce(
                out=tf,
                in_=selall.rearrange("p (c f) -> p c f", f=8),
                axis=mybir.AxisListType.X,
                op=ALU.add,
            )

            for i in range(NCH):
                # final: f32-exact mask on DVE (2x mode), mult on Pool
                msk = opool.tile([128, N], F32, tag="msk", name=f"msk{h}_{i}")
                nc.vector.tensor_scalar(
                    msk, gts[i], tf[:, i : i + 1], None, op0=ALU.is_ge
                )
                ob = obpool.tile([128, N], BF16, tag="ob", name=f"ob{h}_{i}")
                nc.gpsimd.tensor_tensor(out=ob, in0=msk, in1=gts[i], op=ALU.mult)
                nc.sync.dma_start(out[h, i * 128 : (i + 1) * 128, :], ob)

        # software pipeline: produce(h+1) is queued before search(h)
        prev = produce(0)
        for h in range(H):
            cur = produce(h + 1) if h + 1 < H else None
            search(h, *prev)
            prev = cur


def _get_nc():
    global _CACHED_NC
    if _CACHED_NC is None:
        _CACHED_NC = _build_nc()
    return _CACHED_NC


def run(x, W_src, W_tgt, trace=False):
    x = np.ascontiguousarray(np.asarray(x, dtype=np.float32))
    W_src = np.ascontiguousarray(np.asarray(W_src, dtype=np.float32))
    W_tgt = np.ascontiguousarray(np.asarray(W_tgt, dtype=np.float32))
    nc = _get_nc()
    in_maps = [
        {"xb": np.ascontiguousarray(x[b].T), "ws": W_src, "wt": W_tgt}
        for b in range(B)
    ]
    res = run_bass_kernel_spmd(nc, in_maps, list(range(B)), trace=trace)
    out = np.stack([res.results[b]["out"] for b in range(B)], axis=0).astype(np.float32)
    return out, res


def kernel(x, W_src, W_tgt):
    out, _ = run(x, W_src, W_tgt, trace=False)
    return out


# revision 12
# speedup vs baseline: 1.2088x; 1.0389x over previous
"""Trainium2 Bass kernel for DirectedGraphLearner (topk_masking).

One NeuronCore per batch b (8 cores total):
    src = x_b @ W_src        [1024, 256] -> heads [4, 64]
    tgt = x_b @ W_tgt
    adj[h] = src_h @ tgt_h^T [1024, 1024]
    out[h] = gelu(adj) * topk_mask(gelu(adj), k=153, rowwise)

Algorithm (v4), exploiting that the row-wise top-k threshold lands at
adj ~ 5..13 sigma where exact-erf gelu(x) == x in fp32, so gelu never
needs computing and only positives can be kept:

  * The PSUM->SBUF copy applies Relu and a free accum_out, giving
    s+ = sum(relu(adj)) per row.  For near-gaussian rows the top-k
    threshold satisfies t ~= C_T * s+ within +-12%, so a per-row
    bracket [t^(1-DLO), t^(1+DHI)] replaces a fixed one.
  * 1 + NB exact counts on q = bf16(relu(adj)) (DVE 4x-mode
    tensor_scalar+accum at 327ns; two chunk-lanes per head count on ACT
    via Sign+accum) bisect to a bracket holding ~3 candidates; counts
    are exact because trial points are generic f32 values that never
    land on the bf16 grid.
  * Phase 2 needs no window mask: o = [q < hi]*g keeps every value
    below the bracket top, whose r-th largest (r = K - #{q >= hi} <= 8)
    is exactly the k-th largest of the row (bf16 rounding is monotone,
    so the q-mask never splits f32-adjacent values across hi).  One
    max8 + iota rank-select yields the exact f32 threshold.
  * Output support is f32-exact; output values are bf16-rounded (DRAM
    out is bf16, host upcasts).  Casts and final mask-mults run on the
    otherwise idle Pool engine; heads are software-pipelined so the
    produce stage of head h+1 (matmul, copy, cast) is queued ahead of
    head h's search, keeping Pool's in-order queue from blocking casts.
"""

import numpy as np

import concourse.bass as bass
from concourse import bacc
import concourse.mybir as mybir
import concourse.tile as tile
from concourse.bass_utils import run_bass_kernel_spmd

F32 = mybir.dt.float32
BF16 = mybir.dt.bfloat16
ALU = mybir.AluOpType
AF = mybir.ActivationFunctionType

B, N, D, H, HD = 8, 1024, 256, 4, 64
K = 153  # max(1, int(0.15 * 1024))
NCH = N // 128  # row chunks per head

# t ~= C_T * sum(relu(row)); empirical ratio spread -10.2%/+11.8%,
# margins widened ~1.15x.
C_T = 2.548730e-03
DLO = 0.118
DHI = 0.136
NB = 5  # bisection iterations after the bracket-top count

# The bracket-top (chi) count runs entirely on ACT (Sign+accum): it is
# off the sequential bisection chain (only needed at rank-select), so
# ACT absorbs it while DVE runs the latency-critical bisection.
CHI_ON_ACT = True
ACT_CHUNKS = ()  # bisect chunk lanes whose counts run on ACT

_CACHED_NC = None


def _build_nc():
    nc = bacc.Bacc()
    # xb is passed host-side pre-transposed: [D, N] == x[b].T
    xb = nc.declare_dram_parameter("xb", [D, N], F32, isOutput=False)
    ws = nc.declare_dram_parameter("ws", [D, D], F32, isOutput=False)
    wt = nc.declare_dram_parameter("wt", [D, D], F32, isOutput=False)
    out = nc.declare_dram_parameter("out", [H, N, N], BF16, isOutput=True)
    with tile.TileContext(nc) as tc:
        _body(tc, xb, ws, wt, out)
    nc.compile()
    return nc


def _body(tc, xb, ws, wt, out):
    nc = tc.nc
    with (
        tc.tile_pool(name="persist", bufs=1) as ppool,
        tc.tile_pool(name="g", bufs=2) as gpool,
        tc.tile_pool(name="q", bufs=2) as qpool,
        tc.tile_pool(name="o", bufs=3) as opool,
        tc.tile_pool(name="ob", bufs=6) as obpool,
        tc.tile_pool(name="small", bufs=2) as spool,
        tc.tile_pool(name="jnk", bufs=2) as jpool,
        tc.tile_pool(name="ppsum", bufs=2, space="PSUM") as ppsum,
        tc.tile_pool(name="apsum", bufs=3, space="PSUM") as apsum,
    ):
        # ---- load xT [256, 1024] (host passes x[b].T) and weights ----
        xT = [ppool.tile([128, N], F32, tag=f"xT{d}", name=f"xT{d}") for d in range(2)]
        for dh in range(2):
            nc.sync.dma_start(xT[dh], xb[dh * 128 : (dh + 1) * 128, :])
        wst = [ppool.tile([128, D], F32, tag=f"ws{kc}", name=f"wst{kc}") for kc in range(2)]
        wtt = [ppool.tile([128, D], F32, tag=f"wt{kc}", name=f"wtt{kc}") for kc in range(2)]
        for kc in range(2):
            nc.sync.dma_start(wst[kc], ws[kc * 128 : (kc + 1) * 128, :])
            nc.sync.dma_start(wtt[kc], wt[kc * 128 : (kc + 1) * 128, :])

        # ---- projections: srcT/tgtT = (x @ W)^T = W^T x^T, [256, 1024]
        # m=0 tiles (heads 0-1) are produced first so head-0 adj matmuls
        # can start while the m=1 projections still run.
        srcT = [ppool.tile([128, N], F32, tag=f"sT{m}", name=f"srcT{m}") for m in range(2)]
        tgtT = [ppool.tile([128, N], F32, tag=f"tT{m}", name=f"tgtT{m}") for m in range(2)]
        for m in range(2):
            for wtiles, ttiles in ((wst, srcT), (wtt, tgtT)):
                for nh in range(2):
                    pp = ppsum.tile([128, 512], F32, tag="pp")
                    for kc in range(2):
                        nc.tensor.matmul(
                            pp,
                            wtiles[kc][:, m * 128 : (m + 1) * 128],
                            xT[kc][:, nh * 512 : (nh + 1) * 512],
                            start=(kc == 0),
                            stop=(kc == 1),
                        )
                    nc.scalar.copy(ttiles[m][:, nh * 512 : (nh + 1) * 512], pp)

        # iota row 0..7, for rank-select from the max8 output
        iota8 = ppool.tile([128, 8], F32, tag="iota8", name="iota8")
        for j in range(8):
            nc.vector.memset(iota8[:, j : j + 1], float(j))

        def produce(h):
            """adj matmuls + relu-copy (+accum) + bf16 cast for head h."""
            ht = h // 2
            hs = (h % 2) * HD
            sp = spool.tile([128, NCH], F32, tag=f"sp{h % 2}")
            gts, qts = [], []
            for i in range(NCH):
                ap = apsum.tile([128, N], F32, tag="ap")
                for nh in range(2):
                    nc.tensor.matmul(
                        ap[:, nh * 512 : (nh + 1) * 512],
                        srcT[ht][hs : hs + HD, i * 128 : (i + 1) * 128],
                        tgtT[ht][hs : hs + HD, nh * 512 : (nh + 1) * 512],
                    )
                g = gpool.tile([128, N], F32, tag=f"g{i}", name=f"g{h}_{i}")
                nc.scalar.activation(g, ap, AF.Relu, accum_out=sp[:, i : i + 1])
                gts.append(g)
                q = qpool.tile([128, N], BF16, tag=f"q{i}", name=f"q{h}_{i}")
                nc.gpsimd.tensor_copy(q, g)
                qts.append(q)
            return sp, gts, qts

        def search(h, sp, gts, qts):
            """bisection search + extraction + masked store for head h."""
            s = h % 2
            lo = spool.tile([128, NCH], F32, tag=f"lo{s}")
            w0h = spool.tile([128, NCH], F32, tag=f"w0h{s}")
            hi = spool.tile([128, NCH], F32, tag=f"hi{s}")
            tri = spool.tile([128, NCH], F32, tag=f"tri{s}")
            trin = spool.tile([128, NCH], F32, tag=f"trin{s}")
            cnt = spool.tile([128, NCH], F32, tag=f"cnt{s}")
            chi = spool.tile([128, NCH], F32, tag=f"chi{s}")
            pred = spool.tile([128, NCH], F32, tag=f"pred{s}")
            npred = spool.tile([128, NCH], F32, tag=f"npred{s}")
            dl = spool.tile([128, NCH], F32, tag=f"dl{s}")
            that = spool.tile([128, NCH], F32, tag=f"that{s}")

            nc.vector.tensor_scalar(that, sp, float(C_T), None, op0=ALU.mult)
            nc.vector.tensor_scalar(lo, that, float(1.0 - DLO), None, op0=ALU.mult)
            nc.vector.tensor_scalar(w0h, that, float((DLO + DHI) / 2.0), None, op0=ALU.mult)
            # bracket top: hi0 = lo + 2*w0h; chi = #{q >= hi0}
            nc.vector.scalar_tensor_tensor(hi, w0h, 2.0, lo, op0=ALU.mult, op1=ALU.add)
            nc.vector.tensor_scalar(trin, hi, -1.0, None, op0=ALU.mult)
            for i in range(NCH):
                jk = jpool.tile([128, N], BF16, tag=f"jk{i}", name=f"jkc{h}_{i}")
                if CHI_ON_ACT:
                    nc.scalar.activation(
                        jk, qts[i], AF.Sign,
                        bias=trin[:, i : i + 1], accum_out=chi[:, i : i + 1],
                    )
                else:
                    nc.vector.tensor_scalar(
                        jk, qts[i], hi[:, i : i + 1], None,
                        op0=ALU.is_ge, op1=ALU.add, accum_out=chi[:, i : i + 1],
                    )
            if CHI_ON_ACT:
                # ACT returns s = 2*cnt - N; convert
                nc.vector.tensor_scalar(chi, chi, 0.5, float(N / 2.0), op0=ALU.mult, op1=ALU.add)
            for it in range(NB):
                nc.vector.tensor_add(tri, lo, w0h)
                if ACT_CHUNKS:
                    nc.vector.tensor_scalar(trin, tri, -1.0, None, op0=ALU.mult)
                for i in range(NCH):
                    jk = jpool.tile([128, N], BF16, tag=f"jk{i}", name=f"jkb{h}_{it}_{i}")
                    if i in ACT_CHUNKS:
                        nc.scalar.activation(
                            jk, qts[i], AF.Sign,
                            bias=trin[:, i : i + 1], accum_out=cnt[:, i : i + 1],
                        )
                    else:
                        nc.vector.tensor_scalar(
                            jk, qts[i], tri[:, i : i + 1], None,
                            op0=ALU.is_ge, op1=ALU.add, accum_out=cnt[:, i : i + 1],
                        )
                if ACT_CHUNKS:
                    a0 = ACT_CHUNKS[0]
                    nc.vector.tensor_scalar(
                        cnt[:, a0 : a0 + len(ACT_CHUNKS)],
                        cnt[:, a0 : a0 + len(ACT_CHUNKS)],
                        0.5, float(N / 2.0), op0=ALU.mult, op1=ALU.add,
                    )
                nc.vector.tensor_scalar(pred, cnt, float(K), None, op0=ALU.is_ge)
                nc.vector.tensor_scalar(npred, pred, -1.0, 1.0, op0=ALU.mult, op1=ALU.add)
                nc.vector.tensor_mul(dl, pred, w0h)
                nc.vector.tensor_add(lo, lo, dl)
                # chi <- pred ? chi : cnt
                nc.vector.tensor_sub(dl, cnt, chi)
                nc.vector.tensor_mul(dl, dl, npred)
                nc.vector.tensor_add(chi, chi, dl)
                nc.vector.tensor_scalar(w0h, w0h, 0.5, None, op0=ALU.mult)
            nc.vector.scalar_tensor_tensor(hi, w0h, 2.0, lo, op0=ALU.mult, op1=ALU.add)

            # rank among candidates: m1 = clip(K-1 - chi, 0, 7)
            m1 = spool.tile([128, NCH], F32, tag=f"m1{s}")
            tf = spool.tile([128, NCH], F32, tag=f"tf{s}")
            nc.vector.tensor_scalar(m1, chi, -1.0, float(K - 1), op0=ALU.mult, op1=ALU.add)
            nc.vector.tensor_scalar_min(m1, m1, 7.0)
            nc.vector.tensor_scalar_max(m1, m1, 0.0)

            mxall = spool.tile([128, 8 * NCH], F32, tag=f"mxall{s}")
            for i in range(NCH):
                # o = [q < hi] * g  (all values below the bracket top)
                o = opool.tile([128, N], F32, tag="o", name=f"o{h}_{i}")
                nc.vector.scalar_tensor_tensor(
                    o, qts[i], hi[:, i : i + 1], gts[i], op0=ALU.is_lt, op1=ALU.mult
                )
                nc.vector.max(out=mxall[:, 8 * i : 8 * i + 8], in_=o)

            # batched rank-select: tf_i = mxall[i*8 + m1_i]
            selall = spool.tile([128, 8 * NCH], F32, tag=f"selall{s}")
            nc.vector.tensor_tensor(
                out=selall.rearrange("p (c f) -> p c f", f=8),
                in0=m1.rearrange("p (c u) -> p c u", u=1).to_broadcast([128, NCH, 8]),
                in1=iota8.rearrange("p (u f) -> p u f", u=1).to_broadcast([128, NCH, 8]),
                op=ALU.is_equal,
            )
            nc.vector.tensor_tensor(out=selall, in0=selall, in1=mxall, op=ALU.mult)
            nc.vector.tensor_reduce(
                out=tf,
                in_=selall.rearrange("p (c f) -> p c f", f=8),
                axis=mybir.AxisListType.X,
                op=ALU.add,
            )

            for i in range(NCH):
                # final: f32-exact mask on DVE (2x mode, bf16 out), mult on Pool
                msk = opool.tile([128, N], BF16, tag="msk", name=f"msk{h}_{i}")
                nc.vector.tensor_scalar(
                    msk, gts[i], tf[:, i : i + 1], None, op0=ALU.is_ge
                )
                ob = obpool.tile([128, N], BF16, tag="ob", name=f"ob{h}_{i}")
                nc.gpsimd.tensor_tensor(out=ob, in0=msk, in1=gts[i], op=ALU.mult)
                nc.sync.dma_start(out[h, i * 128 : (i + 1) * 128, :], ob)

        # software pipeline: produce(h+1) is queued before search(h)
        prev = produce(0)
        for h in range(H):
            cur = produce(h + 1) if h + 1 < H else None
            search(h, *prev)
            prev = cur


def _get_nc():
    global _CACHED_NC
    if _CACHED_NC is None:
        _CACHED_NC = _build_nc()
    return _CACHED_NC


def run(x, W_src, W_tgt, trace=False):
    x = np.ascontiguousarray(np.asarray(x, dtype=np.float32))
    W_src = np.ascontiguousarray(np.asarray(W_src, dtype=np.float32))
    W_tgt = np.ascontiguousarray(np.asarray(W_tgt, dtype=np.float32))
    nc = _get_nc()
    in_maps = [
        {"xb": np.ascontiguousarray(x[b].T), "ws": W_src, "wt": W_tgt}
        for b in range(B)
    ]
    res = run_bass_kernel_spmd(nc, in_maps, list(range(B)), trace=trace)
    out = np.stack([res.results[b]["out"] for b in range(B)], axis=0).astype(np.float32)
    return out, res


def kernel(x, W_src, W_tgt):
    out, _ = run(x, W_src, W_tgt, trace=False)
    return out
